# revision 23
# baseline (speedup 1.0000x reference)
"""HyperGNN message-passing kernel v2 (nn_Conv_13778255086166) for 8 TRN2 cores.

Reference computation:
    Xp    = X @ W                                   [N, 64]
    Xe_s  = segment_sum(Xp[vertex], edges, E);  cnt = segment_sum(1, edges, E)
    Ze    = (homo / max(cnt,1)) * Xe_s              [E, 64]
    att_s = segment_sum(homo[edges], vertex, N)
    Xv    = segment_sum(Ze[edges], vertex, N) / att_s
    out   = row_l2_normalize(Xp + Xv)

Distribution: incidence lists sharded by vertex range (core k owns nodes
[k*12500, (k+1)*12500)); per-core edge partials AllReduced (bf16).

v2 vs baseline:
  - gathered tables stored bf16 with 256B rows (phase-2 traffic halved)
  - cnt (pure index data) computed on host -> no cnt matmul chain, no val1
  - AllReduce in bf16 (half the wire bytes)
  - one-hot masks built in one batched DVE op per tile (bf16, 2x rate)
  - Xp kept in SBUF for phase 2 (no reload), single big strided DMAs for
    the Ze build instead of 196 small tiles
"""

from dataclasses import dataclass

import numpy as np

import concourse.bacc as bacc
import concourse.mybir as mybir
import concourse.tile as tile
from concourse import bass_utils

F32 = mybir.dt.float32
BF16 = mybir.dt.bfloat16
I16 = mybir.dt.int16


@dataclass(frozen=True)
class Cfg:
    n_cores: int = 8
    N: int = 100000
    E: int = 25000
    cap1: int = 1536   # incidence slots per 128-edge tile per core (mult of 128)
    cap2: int = 3072   # incidence slots per 128-node tile per core (mult of 128)

    @staticmethod
    def from_inputs(vertex, edges, n_cores=8, N=100000, E=25000):
        """Tight per-tile slot caps computed from the actual index data."""
        vertex = np.asarray(vertex).astype(np.int64)
        edges = np.asarray(edges).astype(np.int64)
        npc = N // n_cores
        etiles = ((E + 1 + 127) // 128 * 128) // 128
        ntiles = ((npc + 1 + 127) // 128 * 128) // 128
        m1 = m2 = 1
        for k in range(n_cores):
            sel = (vertex >= k * npc) & (vertex < (k + 1) * npc)
            v_l, e_l = vertex[sel] - k * npc, edges[sel]
            m1 = max(m1, int(np.bincount(e_l >> 7, minlength=etiles).max()))
            m2 = max(m2, int(np.bincount(v_l >> 7, minlength=ntiles).max()))
        r128 = lambda x: max(256, (x + 127) // 128 * 128)
        return Cfg(n_cores=n_cores, N=N, E=E, cap1=r128(m1), cap2=r128(m2))

    @property
    def npc(self):
        assert self.N % self.n_cores == 0
        return self.N // self.n_cores

    @property
    def npcp(self):  # padded, with at least one spare zero row
        return (self.npc + 1 + 127) // 128 * 128

    @property
    def ntiles(self):
        return self.npcp // 128

    @property
    def ep(self):
        return (self.E + 1 + 127) // 128 * 128

    @property
    def etiles(self):
        return self.ep // 128


def _bf16():
    import ml_dtypes
    return ml_dtypes.bfloat16


def wrap_idx(idx: np.ndarray) -> np.ndarray:
    """int16 index layout for dma_gather: element j at [j%16, j//16],
    replicated across the 8 16-partition groups (one per Q7 cpu)."""
    s = idx.shape[0]
    assert s % 16 == 0
    w = np.ascontiguousarray(idx.astype(np.int16).reshape(-1, 16).T)
    return np.tile(w, (8, 1))


def prep_core_inputs(cfg: Cfg, k: int, X, W, homo, vertex, edges):
    """Host-side shard/sort/pad for core k (index/layout reorganization only)."""
    bf16 = _bf16()
    npc, npcp = cfg.npc, cfg.npcp
    vertex = np.asarray(vertex)
    edges = np.asarray(edges)
    sel = (vertex >= k * npc) & (vertex < (k + 1) * npc)
    v_l = (vertex[sel] - k * npc).astype(np.int64)
    e_l = edges[sel].astype(np.int64)

    def build(seg, other, tiles_n, cap, pad_gather):
        o = np.argsort(seg, kind="stable")
        s, g = seg[o], other[o]
        t_of = s >> 7
        counts = np.bincount(t_of, minlength=tiles_n)
        assert (counts <= cap).all(), (counts.max(), cap)
        starts = np.cumsum(counts) - counts
        rank = np.arange(len(s)) - starts[t_of]
        dest = t_of * cap + rank
        S = tiles_n * cap
        gi = np.full(S, pad_gather, np.int64)
        off = np.zeros(S, np.float32)
        gi[dest] = g
        off[dest] = (s & 127).astype(np.float32)
        return gi, off

    # P1: segment by edge, gather by local vertex; pads gather zero row npc.
    g1, off1 = build(e_l, v_l, cfg.etiles, cfg.cap1, pad_gather=npc)
    # P2: segment by local vertex, gather by edge; pads gather zero row E.
    g2, off2 = build(v_l, e_l, cfg.ntiles, cfg.cap2, pad_gather=cfg.E)

    def tilemaj_idx(gi, tiles_n, cap):
        w = np.stack([wrap_idx(gi[t * cap:(t + 1) * cap]) for t in range(tiles_n)])
        return np.ascontiguousarray(w)

    def tilemaj_off(a, tiles_n, cap):
        return np.ascontiguousarray(
            a.reshape(tiles_n, cap // 128, 128).transpose(0, 2, 1)).astype(bf16)

    Xt = np.zeros((64, npcp), np.float32)
    Xt[:, :npc] = np.asarray(X)[k * npc:(k + 1) * npc].T

    homo_pad = np.zeros(cfg.ep, np.float32)
    homo_pad[:cfg.E] = np.asarray(homo)
    homo_t = np.ascontiguousarray(homo_pad.reshape(cfg.etiles, 128).T)

    # global per-edge incidence counts: pure index data -> host computes
    cnt = np.bincount(edges.astype(np.int64), minlength=cfg.E).astype(np.float32)
    cntr_pad = np.zeros(cfg.ep, np.float32)
    cntr_pad[:cfg.E] = 1.0 / np.maximum(cnt, 1.0)
    cntr_t = np.ascontiguousarray(cntr_pad.reshape(cfg.etiles, 128).T)

    iota = np.broadcast_to(np.arange(128, dtype=np.float32),
                           (128, 128)).astype(bf16).copy()

    return {
        "Xt": Xt,
        "W": np.asarray(W, dtype=np.float32),
        "homo_t": homo_t,
        "cntr_t": cntr_t,
        "iota": iota,
        "g1": tilemaj_idx(g1, cfg.etiles, cfg.cap1),
        "off1": tilemaj_off(off1, cfg.etiles, cfg.cap1),
        "g2": tilemaj_idx(g2, cfg.ntiles, cfg.cap2),
        "off2": tilemaj_off(off2, cfg.ntiles, cfg.cap2),
    }


def build_nc(cfg: Cfg, for_sim: bool = False, variant: str = "full",
             repeat: int = 1):
    """variant: full | nocc | p1 | p2 | p1n | p2n | nog (see kernel.py)"""
    from itertools import product
    no_g1 = variant in ("p1n", "nog")
    no_g2 = variant in ("p2n", "nog")
    variant = {"p1n": "p1", "p2n": "p2", "nog": "nocc"}.get(variant, variant)
    c1 = cfg.cap1 // 128
    c2 = cfg.cap2 // 128
    nt, et = cfg.ntiles, cfg.etiles
    nc = bacc.Bacc("TRN2", target_bir_lowering=False, debug=False,
                   num_devices=1 if for_sim else cfg.n_cores,
                   num_swdge_queues=4)

    xt_d = nc.dram_tensor("Xt", [64, cfg.npcp], F32, kind="ExternalInput")
    w_d = nc.dram_tensor("W", [64, 64], F32, kind="ExternalInput")
    homo_d = nc.dram_tensor("homo_t", [128, et], F32, kind="ExternalInput")
    cntr_d = nc.dram_tensor("cntr_t", [128, et], F32, kind="ExternalInput")
    iota_d = nc.dram_tensor("iota", [128, 128], BF16, kind="ExternalInput")
    g1_d = nc.dram_tensor("g1", [et, 128, cfg.cap1 // 16], I16, kind="ExternalInput")
    off1_d = nc.dram_tensor("off1", [et, 128, c1], BF16, kind="ExternalInput")
    g2_d = nc.dram_tensor("g2", [nt, 128, cfg.cap2 // 16], I16, kind="ExternalInput")
    off2_d = nc.dram_tensor("off2", [nt, 128, c2], BF16, kind="ExternalInput")
    out_d = nc.dram_tensor("out", [cfg.npcp, 64], F32, kind="ExternalOutput")

    xp_d = nc.dram_tensor("XpD", [cfg.npcp, 128], BF16, kind="Internal")
    eacc_d = nc.dram_tensor("EaccD", [cfg.ep, 64], BF16, kind="Internal")
    ered_d = nc.dram_tensor("EredD", [cfg.ep, 64], BF16, kind="Internal",
                            addr_space="Shared")
    zef_d = nc.dram_tensor("ZeFD", [cfg.ep, 128], BF16, kind="Internal")

    with tile.TileContext(nc) as tc:
        xtiles = next(d for d in range(min(14, nt), 0, -1) if nt % d == 0)
        xchunk = xtiles * 128  # phase-0 X streamed in nt/xtiles chunks
        zchunk = next(d for d in range(min(49, et), 0, -1) if et % d == 0)
        with (
            tc.tile_pool(name="const", bufs=1) as pc,
            tc.tile_pool(name="xin", bufs=2) as px,
            tc.tile_pool(name="idx", bufs=8) as pidx,
            tc.tile_pool(name="gather", bufs=4) as pg,
            tc.tile_pool(name="onehot", bufs=3) as pm,
            tc.tile_pool(name="ze", bufs=2) as pz,
            tc.tile_pool(name="sbout", bufs=3) as po,
            tc.tile_pool(name="fin", bufs=4) as pf,
            tc.tile_pool(name="psum", bufs=2, space="PSUM") as pp,
        ):
            w_sb = pc.tile([64, 64], F32)
            nc.sync.dma_start(out=w_sb[:], in_=w_d[:])
            iota_sb = pc.tile([128, 128], BF16)
            nc.sync.dma_start(out=iota_sb[:], in_=iota_d[:])
            homo_sb = pc.tile([128, et], F32)
            nc.sync.dma_start(out=homo_sb[:], in_=homo_d[:])
            cntr_sb = pc.tile([128, et], F32)
            nc.sync.dma_start(out=cntr_sb[:], in_=cntr_d[:])

            # persistent SBUF copies built in phase 0
            xp_all = pc.tile([128, nt, 64], F32)    # f32 Xp for phase-2 add
            xq_all = pc.tile([128, nt, 64], BF16)   # bf16 Xp staged for DRAM

            # phase 0: Xp = X_local @ W; stage to SBUF, one strided DMA out
            for rep, cs in product(range(repeat), range(0, cfg.npcp, xchunk)):
                xc = px.tile([64, xchunk], F32, tag="xc")
                nc.sync.dma_start(out=xc[:], in_=xt_d[:, cs:cs + xchunk])
                for u in range(xchunk // 128):
                    t = cs // 128 + u
                    ps = pp.tile([128, 64], F32, tag="ps0")
                    nc.tensor.matmul(ps[:], lhsT=xc[:, u * 128:(u + 1) * 128],
                                     rhs=w_sb[:], start=True, stop=True)
                    nc.vector.tensor_copy(out=xp_all[:, t, :], in_=ps[:])
                    nc.vector.tensor_copy(out=xq_all[:, t, :], in_=ps[:])
                if cs + xchunk == cfg.npcp:
                    xp_view = xp_d.ap().rearrange("(t p) f -> p t f", p=128)
                    nc.sync.dma_start(out=xp_view[:, :, 0:64], in_=xq_all[:])

            # phase 1: edge-tile accumulation of bf16 Xp rows
            p1_reps = 0 if variant in ("p0", "p2") else repeat
            for rep, s in product(range(p1_reps), range(et)):
                gi = pidx.tile([128, cfg.cap1 // 16], I16, tag="gi1")
                nc.sync.dma_start(out=gi[:], in_=g1_d[s])
                of = pidx.tile([128, c1], BF16, tag="of1")
                nc.sync.dma_start(out=of[:], in_=off1_d[s])
                g = pg.tile([128, c1, 128], BF16, tag="g1")
                if no_g1:
                    nc.vector.memset(g[:], 0.0)
                else:
                    ha = (c1 + 1) // 2
                    nc.gpsimd.dma_gather(g[:, 0:ha, :], xp_d[:],
                                         gi[:, 0:ha * 8], ha * 128,
                                         ha * 128, 128, single_packet=False,
                                         queue_num=(2 * s) % 4)
                    nc.gpsimd.dma_gather(g[:, ha:c1, :], xp_d[:],
                                         gi[:, ha * 8:], (c1 - ha) * 128,
                                         (c1 - ha) * 128, 128,
                                         single_packet=False,
                                         queue_num=(2 * s + 1) % 4)
                mt = pm.tile([128, c1, 128], BF16, tag="mt1")
                nc.vector.tensor_tensor(
                    out=mt[:],
                    in0=iota_sb[:].unsqueeze(1).broadcast_to([128, c1, 128]),
                    in1=of[:].unsqueeze(2).broadcast_to([128, c1, 128]),
                    op=mybir.AluOpType.is_equal)
                ps = pp.tile([128, 64], F32, tag="ps1")
                for j in range(c1):
                    nc.tensor.matmul(ps[:], lhsT=mt[:, j, :], rhs=g[:, j, 0:64],
                                     start=(j == 0), stop=(j == c1 - 1))
                acc = po.tile([128, 64], BF16, tag="acc1")
                nc.vector.tensor_copy(out=acc[:], in_=ps[:])
                nc.sync.dma_start(out=eacc_d[s * 128:(s + 1) * 128, :], in_=acc[:])

            # AllReduce edge partials (bf16)
            cc_reps = 0 if variant in ("p0", "p1") else repeat
            for rep in range(cc_reps):
                if for_sim or variant in ("nocc", "p2"):
                    nc.sync.dma_start(out=ered_d[:], in_=eacc_d[:])
                else:
                    nc.gpsimd.collective_compute(
                        "AllReduce", mybir.AluOpType.add,
                        replica_groups=[list(range(cfg.n_cores))],
                        ins=[eacc_d.ap()], outs=[ered_d.ap()],
                    )

            # Ze build: zef rows = [Ye*homo | homo | junk], chunk-batched
            scale_sb = pf.tile([128, et], F32, tag="scale")
            nc.vector.tensor_tensor(out=scale_sb[:], in0=homo_sb[:],
                                    in1=cntr_sb[:], op=mybir.AluOpType.mult)
            er_view = ered_d.ap().rearrange("(t p) f -> p t f", p=128)
            zf_view = zef_d.ap().rearrange("(t p) f -> p t f", p=128)
            for rep, zs in product(range(cc_reps), range(0, et, zchunk)):
                er = pz.tile([128, zchunk, 64], BF16, tag="er")
                nc.sync.dma_start(out=er[:], in_=er_view[:, zs:zs + zchunk, :])
                z = pz.tile([128, zchunk, 128], BF16, tag="z")
                nc.vector.tensor_tensor(
                    out=z[:, :, 0:64], in0=er[:],
                    in1=scale_sb[:, zs:zs + zchunk].unsqueeze(2)
                        .broadcast_to([128, zchunk, 64]),
                    op=mybir.AluOpType.mult)
                nc.vector.tensor_copy(
                    out=z[:, :, 64:65],
                    in_=homo_sb[:, zs:zs + zchunk].unsqueeze(2))
                nc.sync.dma_start(out=zf_view[:, zs:zs + zchunk, :], in_=z[:])

            # phase 2: node-tile accumulation + finalize
            for rep, s in product(range(cc_reps), range(nt)):
                gi = pidx.tile([128, cfg.cap2 // 16], I16, tag="gi2")
                nc.sync.dma_start(out=gi[:], in_=g2_d[s])
                of = pidx.tile([128, c2], BF16, tag="of2")
                nc.sync.dma_start(out=of[:], in_=off2_d[s])
                g = pg.tile([128, c2, 128], BF16, tag="g2")
                if no_g2:
                    nc.vector.memset(g[:], 0.0)
                else:
                    ha = (c2 + 1) // 2
                    nc.gpsimd.dma_gather(g[:, 0:ha, :], zef_d[:],
                                         gi[:, 0:ha * 8], ha * 128,
                                         ha * 128, 128, single_packet=False,
                                         queue_num=(2 * s) % 4)
                    nc.gpsimd.dma_gather(g[:, ha:c2, :], zef_d[:],
                                         gi[:, ha * 8:], (c2 - ha) * 128,
                                         (c2 - ha) * 128, 128,
                                         single_packet=False,
                                         queue_num=(2 * s + 1) % 4)
                mt = pm.tile([128, c2, 128], BF16, tag="mt2")
                nc.vector.tensor_tensor(
                    out=mt[:],
                    in0=iota_sb[:].unsqueeze(1).broadcast_to([128, c2, 128]),
                    in1=of[:].unsqueeze(2).broadcast_to([128, c2, 128]),
                    op=mybir.AluOpType.is_equal)
                ps = pp.tile([128, 65], F32, tag="ps2")
                for j in range(c2):
                    nc.tensor.matmul(ps[:], lhsT=mt[:, j, :], rhs=g[:, j, 0:65],
                                     start=(j == 0), stop=(j == c2 - 1))
                attm = pf.tile([128, 1], F32, tag="attm")
                nc.vector.tensor_scalar_max(out=attm[:], in0=ps[:, 64:65],
                                            scalar1=1e-30)
                arec = pf.tile([128, 1], F32, tag="arec")
                nc.vector.reciprocal(out=arec[:], in_=attm[:])
                o = pf.tile([128, 64], F32, tag="o")
                nc.vector.tensor_scalar_mul(out=o[:], in0=ps[:, 0:64],
                                            scalar1=arec[:])
                nc.vector.tensor_tensor(out=o[:], in0=o[:], in1=xp_all[:, s, :],
                                        op=mybir.AluOpType.add)
                sq = pf.tile([128, 64], F32, tag="sq")
                nc.vector.tensor_tensor(out=sq[:], in0=o[:], in1=o[:],
                                        op=mybir.AluOpType.mult)
                rs = pf.tile([128, 1], F32, tag="rs")
                nc.vector.reduce_sum(out=rs[:], in_=sq[:],
                                     axis=mybir.AxisListType.X)
                rn = pf.tile([128, 1], F32, tag="rn")
                nc.scalar.sqrt(out=rn[:], in_=rs[:])
                rnm = pf.tile([128, 1], F32, tag="rnm")
                nc.vector.tensor_scalar_max(out=rnm[:], in0=rn[:], scalar1=1e-30)
                rrec = pf.tile([128, 1], F32, tag="rrec")
                nc.vector.reciprocal(out=rrec[:], in_=rnm[:])
                ot = po.tile([128, 64], F32, tag="ot")
                nc.vector.tensor_scalar_mul(out=ot[:], in0=o[:], scalar1=rrec[:])
                nc.sync.dma_start(out=out_d[s * 128:(s + 1) * 128, :], in_=ot[:])

    nc.compile()
    return nc


_NC_CACHE = {}


def kernel(**inputs) -> np.ndarray:
    """Full inputs in, full output out. Shards across 8 NeuronCores internally."""
    X = np.asarray(inputs["X"], dtype=np.float32)
    W = np.asarray(inputs["W"], dtype=np.float32)
    homo = np.asarray(inputs["homo"], dtype=np.float32)
    vertex = np.asarray(inputs["vertex"])
    edges = np.asarray(inputs["edges"])
    cfg = Cfg.from_inputs(vertex, edges)
    assert X.shape == (cfg.N, 64) and homo.shape == (cfg.E,)

    key = cfg
    if key not in _NC_CACHE:
        _NC_CACHE[key] = build_nc(cfg)
    nc = _NC_CACHE[key]

    in_maps = [prep_core_inputs(cfg, k, X, W, homo, vertex, edges)
               for k in range(cfg.n_cores)]
    res = bass_utils.run_bass_kernel_spmd(
        nc, in_maps, core_ids=list(range(cfg.n_cores)))
    out = np.concatenate(
        [res.results[k]["out"][:cfg.npc] for k in range(cfg.n_cores)], axis=0)
    return out.astype(np.float32)
